# revision 10
# baseline (speedup 1.0000x reference)
"""DHPF (dynamic high-pass filter) Trainium2 Bass kernel — Toeplitz v6.

Full inputs in, full outputs out. Sharding: pure data parallelism — sample b of
x[8, 64, 256, 256] goes to core b.

Algorithm (per core = 1 sample, 64 channels of 256x256):
  out = | X~ - Csig @ X~ @ Csig |,   X~ = X * e^{i pi (r+c)/256}
  with Csig[r,y] = sigma[y-r] real symmetric Toeplitz (the box-lowpass
  convolution operator with its rank-1 phase folded into the data; see v5).
  X~ is host-side input prep, shipped packed 2-channels-per-DMA. Csig is built
  on device once per sample from the channel-0 box-energy cutoff.
  abs() uses a custom DVE op SQDIFF_ANT: out = (in0-in1)^2.
  PE stream is software-pipelined (stA(i+1) before stB(i)); constants arrive
  in two packed DMAs; the cutoff scalar chain runs broadcast on [128,1] to
  avoid cross-engine round-trips.
"""

import sys
import types

import numpy as np

# The agent image's antenv is a stub without axon_hooks; rebuild the NTFF
# profile hook so trace=True (HW exec time) is available when requested.
try:
    if "antenv.axon_hooks" not in sys.modules:
        from trn_agent_boot.trn_boot import _ntff_profile_via_ctypes

        _hooks = types.ModuleType("antenv.axon_hooks")
        _h = _ntff_profile_via_ctypes("/opt/axon/libaxon_pjrt.so")
        _hooks.get_axon_ntff_profile_hook = lambda: _h
        _hooks.set_axon_ntff_profile_hook = lambda h: None
        sys.modules["antenv.axon_hooks"] = _hooks
except Exception:
    pass

import concourse.bass as bass
import concourse.tile as tile
from concourse import bacc, mybir
from concourse import bass_utils
from concourse.bass import ds, ts
from concourse.bass_utils import run_bass_kernel_spmd

try:
    bass_utils.upload_artifacts = lambda tmpdir: tmpdir
except Exception:
    pass

f32 = mybir.dt.float32
f16 = mybir.dt.float16
ALU = mybir.AluOpType

N = 256
CH = 64
ENERGY = 0.4


# ---------------- custom DVE op: out = (in0 - in1)^2 ----------------------
def _register_sqdiff():
    import concourse.dve_ops as dom
    from concourse.dve_spec import Spec, Src0, Src1, sq, lower, _has_src1
    from concourse.dve_uop import DveOpSpec

    name = "SQDIFF_ANT"
    for op in dom.OPS:
        if op.name == name:
            return op
    from concourse.dve_spec import C0
    spec = Spec(
        body=sq(Src0 * C0 - Src1),
        reference=lambda in0, in1, s0, s1, imm2: (
            (in0.astype(np.float32) * s0 - in1.astype(np.float32)) ** 2
        ).astype(np.float32),
    )
    opcode = dom._CUSTOM_DVE_ROW_BASE + len(dom.OPS)
    shas = {}
    for ver in ("v3", "v4"):
        try:
            d = DveOpSpec(
                name=name, opcode=opcode, uops=lower(spec, ver=ver),
                rd1_en=_has_src1(spec),
            )
            shas[ver] = d.sha(ver)
        except Exception:
            pass
    op = dom.DveOp(name, spec, subdim=False, uops_sha=shas)
    dom.OPS.append(op)
    dom.CUSTOM_DVE_SPECS[name] = spec
    dom._SUB_OPCODE_FOR_NAME[name] = opcode
    return op


SQDIFF = _register_sqdiff()


def _pack_rows(m):
    """[256, X] -> [128, 2X] in the _split layout (row r = i*128+p)."""
    return np.ascontiguousarray(
        np.stack([m[0:128], m[128:256]], axis=1).reshape(128, -1)
    )


def _host_constants() -> dict[str, np.ndarray]:
    u = np.arange(N)
    D = np.exp(-2j * np.pi * np.outer(u, u) / N)
    S = np.zeros((N, N))
    S[u, (u + N // 2) % N] = 1.0
    A = S @ D
    At = A.T  # [r, u]
    Atr, Ati = At.real, At.imag

    def pack(M1, M2, par):
        return np.concatenate(
            [M1[:128, par::2], M2[:128, par::2]], axis=1
        ).astype(np.float16)

    cabf = np.concatenate([Atr, Ati], axis=1)  # [256, 512]

    crow = N // 2
    dr = np.arange(N) - crow
    mr = np.maximum(-dr, dr + 1).astype(np.float64)
    cids = np.arange(128) + 1
    rmat = (mr[:, None] <= cids[None, :]).astype(np.float64)  # [256, 128]
    ctm = (mr[None, :] <= cids[:, None]).astype(np.float64)  # [128, 256]
    # scrambled-column version: col' = par*128 + j holds v = 2j + par
    ctmp = np.empty_like(ctm)
    jj = np.arange(128)
    for par in (0, 1):
        ctmp[:, par * 128 + jj] = ctm[:, 2 * jj + par]

    # g = mconT^T @ cum: g[p] = ENERGY*cum[127] - cum[p]
    mconT = -np.eye(128)
    mconT[127, :] += ENERGY
    onesJ = np.ones((128, 128))
    mrowc = np.stack([mr[0:128], mr[128:256]], axis=1)  # [128, 2]

    ph = np.pi * np.outer(2 * u + 1, np.arange(N)) / N
    er = np.cos(ph) / 16.0
    ei = -np.sin(ph) / 16.0

    cabfp = _pack_rows(cabf).astype(np.float16)  # [128, 1024]
    cf16 = np.concatenate(
        [
            pack(Atr, Ati, 0), pack(Atr, Ati, 1),
            pack(-Ati, Atr, 0), pack(-Ati, Atr, 1),  # 4 x [128, 256]
            _pack_rows(er),  # [128, 512]
            _pack_rows(ei),  # [128, 512]
            _pack_rows(rmat),  # [128, 256]
        ],
        axis=1,
    ).astype(np.float16)  # [128, 2304]
    cf32 = np.concatenate(
        [
            ctmp,  # [128, 256]
            mconT,  # [128, 128]
            onesJ,  # [128, 128]
            mrowc,  # [128, 2]
        ],
        axis=1,
    ).astype(np.float32)  # [128, 514]
    return {"cabfp": cabfp, "cf16": cf16, "cf32": cf32}


def _host_phase_tables():
    rc = np.pi * (np.arange(N)[:, None] + np.arange(N)[None, :]) / N
    return np.cos(rc).astype(np.float32), np.sin(rc).astype(np.float32)


def _split(t):
    """View a [256, X] dram AP as [128, 2, X] (partition, k-tile, free)."""
    return t.rearrange("(i p) j -> p i j", p=128)


def _build_nc():
    nc = bacc.Bacc("TRN2", target_bir_lowering=False, debug=False)

    xtw = nc.dram_tensor("xtw", [CH // 2, N, 1024], f16, kind="ExternalInput").ap()
    x0 = nc.dram_tensor("x0", [N, N], f16, kind="ExternalInput").ap()
    d_cabf = nc.dram_tensor("cabfp", [128, 1024], f16, kind="ExternalInput").ap()
    d_cf16 = nc.dram_tensor("cf16", [128, 2304], f16, kind="ExternalInput").ap()
    d_cf32 = nc.dram_tensor("cf32", [128, 514], f32, kind="ExternalInput").ap()
    out = nc.dram_tensor("out", [CH, N, N], f32, kind="ExternalOutput").ap()

    with tile.TileContext(nc) as tc:
        with (
            tc.tile_pool(name="consts", bufs=1) as consts,
            tc.tile_pool(name="xp_", bufs=6) as xpool,
            tc.tile_pool(name="pp16", bufs=6) as pp16,
            tc.tile_pool(name="sqp", bufs=6) as sqp,
            tc.tile_pool(name="op", bufs=4) as op,
            tc.tile_pool(name="scratch", bufs=1) as scratch,
            tc.tile_pool(name="pp", bufs=4, space="PSUM") as pp,
        ):
            # ---- gating DMAs first: x0, packed consts ----
            xz = scratch.tile([128, 2, N], f16, tag="xz")
            nc.sync.dma_start(xz[:], _split(x0))
            cabft = consts.tile([128, 1024], f16, tag="cabft")
            nc.sync.dma_start(cabft[:], d_cabf[:, :])
            cf16 = consts.tile([128, 2304], f16, tag="cf16")
            nc.sync.dma_start(cf16[:], d_cf16[:, :])
            cf32 = consts.tile([128, 514], f32, tag="cf32")
            nc.sync.dma_start(cf32[:], d_cf32[:, :])

            cabf = cabft[:].rearrange("p (i j) -> p i j", i=2)
            C16 = {
                "ab1e": cf16[:, ds(0, 256)],
                "ab1o": cf16[:, ds(256, 256)],
                "ab2e": cf16[:, ds(512, 256)],
                "ab2o": cf16[:, ds(768, 256)],
            }
            er = cf16[:, ds(1024, 512)].rearrange("p (i j) -> p i j", i=2)
            ei = cf16[:, ds(1536, 512)].rearrange("p (i j) -> p i j", i=2)
            rmat = cf16[:, ds(2048, 256)].rearrange("p (i j) -> p i j", i=2)
            ctmp = cf32[:, ds(0, 256)]
            mconT = cf32[:, ds(256, 128)]
            onesJ = cf32[:, ds(384, 128)]
            mrowc = cf32[:, ds(512, 2)]

            x_tiles: dict[int, object] = {}

            def load_pair(pr):
                if pr >= CH // 2:
                    return
                t = xpool.tile([128, 2, 1024], f16, tag="x")
                nc.sync.dma_start(t[:], _split(xtw[pr]))
                x_tiles[pr] = t

            for pr in range(3):
                load_pair(pr)

            # ---- PE warmup: ramp the clock while input DMAs are in flight
            warm = scratch.tile([128, 512], f16, tag="warm")
            nc.gpsimd.memset(warm[:], 0.0)
            zer4 = warm[:].rearrange("p (i j) -> p i j", i=4)
            # ============ cutoff from channel 0 (parity forward DFT) ======
            ps1 = pp.tile([128, 2, 512], f32, tag="ps")
            for m in (0, 1):
                for k in (0, 1):
                    nc.tensor.matmul(
                        ps1[:, m, :],
                        lhsT=xz[:, k, ts(m, 128)],
                        rhs=cabf[:, k, :],
                        start=(k == 0),
                        stop=(k == 1),
                    )
            lo2 = scratch.tile([128, 512], f16, tag="utlo")
            nc.scalar.mul(lo2[:], ps1[:, 0, :], 2.0)
            utp = scratch.tile([128, 512], f16, tag="utp")
            nc.vector.scalar_tensor_tensor(
                out=utp[:], in0=lo2[:], scalar=0.5, in1=ps1[:, 1, :],
                op0=ALU.mult, op1=ALU.add,
            )
            utm = scratch.tile([128, 512], f16, tag="utm")
            nc.gpsimd.tensor_sub(utm[:], lo2[:], utp[:])

            ps0 = pp.tile([128, 4, 256], f32, tag="ps")
            for m in (0, 1):
                for par, src in ((0, utp), (1, utm)):
                    e = "e" if par == 0 else "o"
                    sl_re = src[:, ts(m, 128)]
                    sl_im = src[:, ds(256 + m * 128, 128)]
                    nc.tensor.matmul(
                        ps0[:, 2 * m + par, :], lhsT=sl_re, rhs=C16["ab1" + e],
                        start=True, stop=False,
                    )
                    nc.tensor.matmul(
                        ps0[:, 2 * m + par, :], lhsT=sl_im, rhs=C16["ab2" + e],
                        start=False, stop=True,
                    )

            # mag^2 directly on the (column-scrambled) spectrum psum
            mg1 = scratch.tile([128, 4, 128], f16, tag="mg1")
            nc.scalar.activation(
                mg1[:], ps0[:, :, 0:128],
                mybir.ActivationFunctionType.Square, 0.0, 1.0 / 128.0,
            )
            mg2 = scratch.tile([128, 4, 128], f16, tag="mg2")
            nc.vector._custom_dve(
                SQDIFF, out=mg2[:], in0=ps0[:, :, 128:256],
                in1=zer4, s0=1.0 / 128.0,
            )
            mag2 = scratch.tile([128, 4, 128], f16, tag="mag2")
            nc.gpsimd.tensor_add(mag2[:], mg1[:], mg2[:])

            ps_z = pp.tile([128, 2, 256], f32, tag="ps")
            for k in (0, 1):
                nc.tensor.matmul(
                    ps_z[:, 0, :], lhsT=rmat[:, k, :],
                    rhs=mag2[:, 2 * k : 2 * k + 2, :],
                    start=(k == 0), stop=(k == 1),
                )

            wsc = scratch.tile([128, N], f32, tag="wsc")
            cum = scratch.tile([128, 1], f32, tag="cum")
            nc.vector.scalar_tensor_tensor(
                out=wsc[:], in0=ps_z[:, 0, :], scalar=1.0, in1=ctmp,
                op0=ALU.mult, op1=ALU.mult, accum_out=cum[:],
            )
            # g[p] = ENERGY*cum[127] - cum[p]; fail = g > 0; nfb = sum(fail)
            ps_g = pp.tile([128, 2, 256], f32, tag="ps")
            nc.tensor.matmul(
                ps_g[:, 0, 0:1], lhsT=mconT, rhs=cum[:], start=True, stop=True
            )
            fail = scratch.tile([128, 1], f32, tag="fail")
            nc.vector.tensor_scalar(fail[:], ps_g[:, 0, 0:1], 0.0, None, ALU.is_gt)
            ps_nf = pp.tile([128, 2, 256], f32, tag="ps")
            nc.tensor.matmul(
                ps_nf[:, 0, 0:1], lhsT=onesJ, rhs=fail[:], start=True, stop=True
            )
            isok = scratch.tile([128, 1], f32, tag="isok")
            nc.vector.tensor_scalar(
                isok[:], ps_nf[:, 0, 0:1], 126.5, None, ALU.is_le
            )
            tm4 = scratch.tile([128, 1], f32, tag="tm4")
            nc.vector.tensor_scalar(
                tm4[:], ps_nf[:, 0, 0:1], 4.0, None, ALU.subtract
            )
            tsel = scratch.tile([128, 1], f32, tag="tsel")
            nc.vector.tensor_mul(tsel[:], tm4[:], isok[:])
            cutoffb = scratch.tile([128, 1], f32, tag="cutoffb")
            nc.vector.tensor_scalar(cutoffb[:], tsel[:], 5.0, None, ALU.add)
            inrowc = scratch.tile([128, 2], f32, tag="inrowc")
            nc.vector.tensor_scalar(
                inrowc[:], mrowc, cutoffb[:], None, ALU.is_le
            )

            # ====== build Csig = Er^T diag(w) Er + Ei^T diag(w) Ei ======
            # w[u] = inrow[(u+128)%256]: u-half 0 scales by inrow half 1.
            erw = scratch.tile([128, 2, N], f16, tag="erw")
            eiw = scratch.tile([128, 2, N], f16, tag="eiw")
            for hu in (0, 1):
                wsl = inrowc[:, 1 - hu : 2 - hu]
                nc.scalar.mul(erw[:, hu, :], er[:, hu, :], wsl)
                nc.vector.tensor_scalar(
                    eiw[:, hu, :], ei[:, hu, :], wsl, None, ALU.mult
                )
            csig = consts.tile([128, 2, N], f16, tag="csig")
            for hr in (0, 1):
                ps_c = pp.tile([128, 2, 256], f32, tag="ps")
                first = True
                for src in (erw, eiw):
                    base = er if src is erw else ei
                    for hu in (0, 1):
                        nc.tensor.matmul(
                            ps_c[:, 0, :],
                            lhsT=src[:, hu, ts(hr, 128)],
                            rhs=base[:, hu, :],
                            start=first,
                            stop=(src is eiw and hu == 1),
                        )
                        first = False
                nc.scalar.copy(csig[:, hr, :], ps_c[:, 0, :])

            # ============ main loop: out = |X~ - Csig X~ Csig| ============
            def stA(ch):
                """P = Csig @ X~ (both complex parts), psum [128, 2, 512]."""
                xw = x_tiles[ch // 2]
                c = 512 * (ch & 1)
                ps_p = pp.tile([128, 2, 512], f32, tag="ps")
                for m in (0, 1):
                    for part in (0, 1):
                        for hu in (0, 1):
                            nc.tensor.matmul(
                                ps_p[:, m, ds(256 * part, 256)],
                                lhsT=xw[:, hu, ds(c + 256 * part + m * 128, 128)],
                                rhs=csig[:, hu, :],
                                start=(hu == 0),
                                stop=(hu == 1),
                            )
                p16 = pp16.tile([128, 2, 512], f16, tag="p16")
                if ch % 4 == 3:
                    nc.vector.tensor_copy(p16[:, 0, :], ps_p[:, 0, :])
                    nc.vector.tensor_copy(p16[:, 1, :], ps_p[:, 1, :])
                else:
                    nc.scalar.copy(p16[:, 0, :], ps_p[:, 0, :])
                    nc.scalar.copy(p16[:, 1, :], ps_p[:, 1, :])
                return p16

            o_tiles: dict[int, object] = {}

            def stB_abs(ch, p16):
                """Z = P @ Csig; out = sqrt((X~r-Zr)^2 + (X~i-Zi)^2)."""
                xw = x_tiles[ch // 2]
                c = 512 * (ch & 1)
                ps_q = pp.tile([128, 2, 512], f32, tag="ps")
                for my in (0, 1):
                    for part in (0, 1):
                        for mb in (0, 1):
                            nc.tensor.matmul(
                                ps_q[:, my, ds(256 * part, 256)],
                                lhsT=p16[:, mb, ds(256 * part + my * 128, 128)],
                                rhs=csig[:, mb, :],
                                start=(mb == 0),
                                stop=(mb == 1),
                            )
                a = sqp.tile([128, 2, N], f32, tag="a")
                nc.vector._custom_dve(
                    SQDIFF, out=a[:], in0=ps_q[:, :, 0:256],
                    in1=xw[:, :, ds(c, 256)], s0=1.0,
                )
                b = sqp.tile([128, 2, N], f32, tag="b")
                nc.vector._custom_dve(
                    SQDIFF, out=b[:], in0=ps_q[:, :, 256:512],
                    in1=xw[:, :, ds(c + 256, 256)], s0=1.0,
                )
                s = sqp.tile([128, 2, N], f32, tag="s")
                nc.gpsimd.tensor_add(s[:], a[:], b[:])
                if ch & 1 == 0:
                    ot = op.tile([128, 2, 2, N], f32, tag="o")
                    o_tiles[ch // 2] = ot
                o2 = o_tiles[ch // 2]
                nc.scalar.sqrt(o2[:, ch & 1, :, :], s[:])
                if ch & 1:
                    pr = ch // 2
                    x_tiles.pop(pr)
                    orows = out[2 * pr : 2 * pr + 2].rearrange(
                        "c (m p) x -> p c m x", p=128
                    )
                    nc.sync.dma_start(orows, o_tiles.pop(pr)[:])

            p16s: dict[int, object] = {}
            pfirst = stA(0)
            p16s[0] = pfirst
            for i in range(CH):
                if i % 2 == 0:
                    load_pair(i // 2 + 3)
                if i + 1 < CH:
                    pnext = stA(i + 1)
                    p16s[i + 1] = pnext
                stB_abs(i, p16s.pop(i))

    nc.compile()
    return nc


_CACHE: dict[str, object] = {}


def _get_nc():
    if "nc" not in _CACHE:
        _CACHE["nc"] = _build_nc()
    return _CACHE["nc"]


def _get_consts():
    if "consts" not in _CACHE:
        _CACHE["consts"] = _host_constants()
    return _CACHE["consts"]


def _run(x: np.ndarray, trace: bool = False):
    nc = _get_nc()
    consts = _get_consts()
    cph, sph = _host_phase_tables()
    in_maps = []
    for b in range(x.shape[0]):
        xb = np.asarray(x[b], dtype=np.float32)
        xtw = np.empty((CH, N, 512), dtype=np.float16)
        xtw[:, :, 0:256] = (xb * cph[None]).astype(np.float16)
        xtw[:, :, 256:512] = (xb * sph[None]).astype(np.float16)
        m = {
            "xtw": np.ascontiguousarray(
                xtw.reshape(CH // 2, 2, N, 512).transpose(0, 2, 1, 3).reshape(
                    CH // 2, N, 1024
                )
            ),
            "x0": xb[0].astype(np.float16),
        }
        m.update(consts)
        in_maps.append(m)
    res = run_bass_kernel_spmd(
        nc, in_maps, core_ids=list(range(len(in_maps))), trace=trace
    )
    out = np.stack([r["out"] for r in res.results]).astype(np.float32)
    return out, res


def kernel(x: np.ndarray) -> np.ndarray:
    x = np.asarray(x)
    out, _ = _run(x, trace=False)
    return out


# revision 11
# speedup vs baseline: 1.1144x; 1.1144x over previous
"""DHPF (dynamic high-pass filter) Trainium2 Bass kernel — Toeplitz v6.

Full inputs in, full outputs out. Sharding: pure data parallelism — sample b of
x[8, 64, 256, 256] goes to core b.

Algorithm (per core = 1 sample, 64 channels of 256x256):
  out = | X~ - Csig @ X~ @ Csig |,   X~ = X * e^{i pi (r+c)/256}
  with Csig[r,y] = sigma[y-r] real symmetric Toeplitz (the box-lowpass
  convolution operator with its rank-1 phase folded into the data; see v5).
  X~ is host-side input prep, shipped packed 2-channels-per-DMA. Csig is built
  on device once per sample from the channel-0 box-energy cutoff.
  abs() uses a custom DVE op SQDIFF_ANT: out = (in0-in1)^2.
  PE stream is software-pipelined (stA(i+1) before stB(i)); constants arrive
  in two packed DMAs; the cutoff scalar chain runs broadcast on [128,1] to
  avoid cross-engine round-trips.
"""

import sys
import types

import numpy as np

# The agent image's antenv is a stub without axon_hooks; rebuild the NTFF
# profile hook so trace=True (HW exec time) is available when requested.
try:
    if "antenv.axon_hooks" not in sys.modules:
        from trn_agent_boot.trn_boot import _ntff_profile_via_ctypes

        _hooks = types.ModuleType("antenv.axon_hooks")
        _h = _ntff_profile_via_ctypes("/opt/axon/libaxon_pjrt.so")
        _hooks.get_axon_ntff_profile_hook = lambda: _h
        _hooks.set_axon_ntff_profile_hook = lambda h: None
        sys.modules["antenv.axon_hooks"] = _hooks
except Exception:
    pass

import concourse.bass as bass
import concourse.tile as tile
from concourse import bacc, mybir
from concourse import bass_utils
from concourse.bass import ds, ts
from concourse.bass_utils import run_bass_kernel_spmd

try:
    bass_utils.upload_artifacts = lambda tmpdir: tmpdir
except Exception:
    pass

f32 = mybir.dt.float32
f16 = mybir.dt.float16
ALU = mybir.AluOpType

N = 256
CH = 64
ENERGY = 0.4


# ---------------- custom DVE op: out = (in0 - in1)^2 ----------------------
def _register_sqdiff():
    import concourse.dve_ops as dom
    from concourse.dve_spec import Spec, Src0, Src1, sq, lower, _has_src1
    from concourse.dve_uop import DveOpSpec

    name = "SQDIFF_ANT"
    for op in dom.OPS:
        if op.name == name:
            return op
    from concourse.dve_spec import C0
    spec = Spec(
        body=sq(Src0 * C0 - Src1),
        reference=lambda in0, in1, s0, s1, imm2: (
            (in0.astype(np.float32) * s0 - in1.astype(np.float32)) ** 2
        ).astype(np.float32),
    )
    opcode = dom._CUSTOM_DVE_ROW_BASE + len(dom.OPS)
    shas = {}
    for ver in ("v3", "v4"):
        try:
            d = DveOpSpec(
                name=name, opcode=opcode, uops=lower(spec, ver=ver),
                rd1_en=_has_src1(spec),
            )
            shas[ver] = d.sha(ver)
        except Exception:
            pass
    op = dom.DveOp(name, spec, subdim=False, uops_sha=shas)
    dom.OPS.append(op)
    dom.CUSTOM_DVE_SPECS[name] = spec
    dom._SUB_OPCODE_FOR_NAME[name] = opcode
    return op


SQDIFF = _register_sqdiff()


def _pack_rows(m):
    """[256, X] -> [128, 2X] in the _split layout (row r = i*128+p)."""
    return np.ascontiguousarray(
        np.stack([m[0:128], m[128:256]], axis=1).reshape(128, -1)
    )


def _host_constants() -> dict[str, np.ndarray]:
    u = np.arange(N)
    D = np.exp(-2j * np.pi * np.outer(u, u) / N)
    S = np.zeros((N, N))
    S[u, (u + N // 2) % N] = 1.0
    A = S @ D
    At = A.T  # [r, u]
    Atr, Ati = At.real, At.imag

    def pack(M1, M2, par):
        return np.concatenate(
            [M1[:128, par::2], M2[:128, par::2]], axis=1
        ).astype(np.float16)

    cabf = np.concatenate([Atr, Ati], axis=1)  # [256, 512]

    crow = N // 2
    dr = np.arange(N) - crow
    mr = np.maximum(-dr, dr + 1).astype(np.float64)
    cids = np.arange(128) + 1
    rmat = (mr[:, None] <= cids[None, :]).astype(np.float64)  # [256, 128]
    ctm = (mr[None, :] <= cids[:, None]).astype(np.float64)  # [128, 256]
    # scrambled-column version: col' = par*128 + j holds v = 2j + par
    ctmp = np.empty_like(ctm)
    jj = np.arange(128)
    for par in (0, 1):
        ctmp[:, par * 128 + jj] = ctm[:, 2 * jj + par]

    # g = mconT^T @ cum: g[p] = ENERGY*cum[127] - cum[p]
    mconT = -np.eye(128)
    mconT[127, :] += ENERGY
    onesJ = np.ones((128, 128))
    mrowc = np.stack([mr[0:128], mr[128:256]], axis=1)  # [128, 2]

    ph = np.pi * np.outer(2 * u + 1, np.arange(N)) / N
    er = np.cos(ph) / 16.0
    ei = -np.sin(ph) / 16.0

    cabfp = _pack_rows(cabf).astype(np.float16)  # [128, 1024]
    cf16 = np.concatenate(
        [
            pack(Atr, Ati, 0), pack(Atr, Ati, 1),
            pack(-Ati, Atr, 0), pack(-Ati, Atr, 1),  # 4 x [128, 256]
            _pack_rows(er),  # [128, 512]
            _pack_rows(ei),  # [128, 512]
            _pack_rows(rmat),  # [128, 256]
        ],
        axis=1,
    ).astype(np.float16)  # [128, 2304]
    cf32 = np.concatenate(
        [
            ctmp,  # [128, 256]
            mconT,  # [128, 128]
            onesJ,  # [128, 128]
            mrowc,  # [128, 2]
        ],
        axis=1,
    ).astype(np.float32)  # [128, 514]
    return {"cabfp": cabfp, "cf16": cf16, "cf32": cf32}


def _host_phase_tables():
    rc = np.pi * (np.arange(N)[:, None] + np.arange(N)[None, :]) / N
    return np.cos(rc).astype(np.float32), np.sin(rc).astype(np.float32)


def _split(t):
    """View a [256, X] dram AP as [128, 2, X] (partition, k-tile, free)."""
    return t.rearrange("(i p) j -> p i j", p=128)


def _build_nc():
    nc = bacc.Bacc("TRN2", target_bir_lowering=False, debug=False)

    xtw = nc.dram_tensor("xtw", [CH // 2, N, 1024], f16, kind="ExternalInput").ap()
    x0 = nc.dram_tensor("x0", [N, N], f16, kind="ExternalInput").ap()
    d_cabf = nc.dram_tensor("cabfp", [128, 1024], f16, kind="ExternalInput").ap()
    d_cf16 = nc.dram_tensor("cf16", [128, 2304], f16, kind="ExternalInput").ap()
    d_cf32 = nc.dram_tensor("cf32", [128, 514], f32, kind="ExternalInput").ap()
    out = nc.dram_tensor("out", [CH, N, N], f32, kind="ExternalOutput").ap()

    with tile.TileContext(nc) as tc:
        with (
            tc.tile_pool(name="consts", bufs=1) as consts,
            tc.tile_pool(name="xp_", bufs=6) as xpool,
            tc.tile_pool(name="pp16", bufs=6) as pp16,
            tc.tile_pool(name="sqp", bufs=6) as sqp,
            tc.tile_pool(name="op", bufs=4) as op,
            tc.tile_pool(name="scratch", bufs=1) as scratch,
            tc.tile_pool(name="pp", bufs=4, space="PSUM") as pp,
        ):
            # ---- gating DMAs first: x0, packed consts ----
            xz = scratch.tile([128, 2, N], f16, tag="xz")
            nc.sync.dma_start(xz[:], _split(x0))
            cabft = consts.tile([128, 1024], f16, tag="cabft")
            nc.sync.dma_start(cabft[:], d_cabf[:, :])
            cf16 = consts.tile([128, 2304], f16, tag="cf16")
            nc.sync.dma_start(cf16[:], d_cf16[:, :])
            cf32 = consts.tile([128, 514], f32, tag="cf32")
            nc.sync.dma_start(cf32[:], d_cf32[:, :])

            cabf = cabft[:].rearrange("p (i j) -> p i j", i=2)
            C16 = {
                "ab1e": cf16[:, ds(0, 256)],
                "ab1o": cf16[:, ds(256, 256)],
                "ab2e": cf16[:, ds(512, 256)],
                "ab2o": cf16[:, ds(768, 256)],
            }
            er = cf16[:, ds(1024, 512)].rearrange("p (i j) -> p i j", i=2)
            ei = cf16[:, ds(1536, 512)].rearrange("p (i j) -> p i j", i=2)
            rmat = cf16[:, ds(2048, 256)].rearrange("p (i j) -> p i j", i=2)
            ctmp = cf32[:, ds(0, 256)]
            mconT = cf32[:, ds(256, 128)]
            onesJ = cf32[:, ds(384, 128)]
            mrowc = cf32[:, ds(512, 2)]

            x_tiles: dict[int, object] = {}

            def load_pair(pr):
                if pr >= CH // 2:
                    return
                t = xpool.tile([128, 2, 1024], f16, tag="x")
                nc.sync.dma_start(t[:], _split(xtw[pr]))
                x_tiles[pr] = t

            for pr in range(3):
                load_pair(pr)

            # ---- PE warmup: ramp the clock while input DMAs are in flight
            warm = scratch.tile([128, 512], f16, tag="warm")
            nc.gpsimd.memset(warm[:], 0.0)
            zer4 = warm[:].rearrange("p (i j) -> p i j", i=4)
            # ============ cutoff from channel 0 (parity forward DFT) ======
            ps1 = pp.tile([128, 2, 512], f32, tag="ps")
            for m in (0, 1):
                for k in (0, 1):
                    nc.tensor.matmul(
                        ps1[:, m, :],
                        lhsT=xz[:, k, ts(m, 128)],
                        rhs=cabf[:, k, :],
                        start=(k == 0),
                        stop=(k == 1),
                    )
            lo2 = scratch.tile([128, 512], f16, tag="utlo")
            nc.scalar.mul(lo2[:], ps1[:, 0, :], 2.0)
            utp = scratch.tile([128, 512], f16, tag="utp")
            nc.vector.scalar_tensor_tensor(
                out=utp[:], in0=lo2[:], scalar=0.5, in1=ps1[:, 1, :],
                op0=ALU.mult, op1=ALU.add,
            )
            utm = scratch.tile([128, 512], f16, tag="utm")
            nc.gpsimd.tensor_sub(utm[:], lo2[:], utp[:])

            ps0 = pp.tile([128, 4, 256], f32, tag="ps")
            for m in (0, 1):
                for par, src in ((0, utp), (1, utm)):
                    e = "e" if par == 0 else "o"
                    sl_re = src[:, ts(m, 128)]
                    sl_im = src[:, ds(256 + m * 128, 128)]
                    nc.tensor.matmul(
                        ps0[:, 2 * m + par, :], lhsT=sl_re, rhs=C16["ab1" + e],
                        start=True, stop=False,
                    )
                    nc.tensor.matmul(
                        ps0[:, 2 * m + par, :], lhsT=sl_im, rhs=C16["ab2" + e],
                        start=False, stop=True,
                    )

            # mag^2 directly on the (column-scrambled) spectrum psum
            mg1 = scratch.tile([128, 4, 128], f16, tag="mg1")
            nc.scalar.activation(
                mg1[:], ps0[:, :, 0:128],
                mybir.ActivationFunctionType.Square, 0.0, 1.0 / 128.0,
            )
            mg2 = scratch.tile([128, 4, 128], f16, tag="mg2")
            nc.vector._custom_dve(
                SQDIFF, out=mg2[:], in0=ps0[:, :, 128:256],
                in1=zer4, s0=1.0 / 128.0,
            )
            mag2 = scratch.tile([128, 4, 128], f16, tag="mag2")
            nc.gpsimd.tensor_add(mag2[:], mg1[:], mg2[:])

            ps_z = pp.tile([128, 2, 256], f32, tag="ps")
            for k in (0, 1):
                nc.tensor.matmul(
                    ps_z[:, 0, :], lhsT=rmat[:, k, :],
                    rhs=mag2[:, 2 * k : 2 * k + 2, :],
                    start=(k == 0), stop=(k == 1),
                )

            wsc = scratch.tile([128, N], f32, tag="wsc")
            cum = scratch.tile([128, 1], f32, tag="cum")
            nc.vector.scalar_tensor_tensor(
                out=wsc[:], in0=ps_z[:, 0, :], scalar=1.0, in1=ctmp,
                op0=ALU.mult, op1=ALU.mult, accum_out=cum[:],
            )
            # g[p] = ENERGY*cum[127] - cum[p]; fail = g > 0; nfb = sum(fail)
            ps_g = pp.tile([128, 2, 256], f32, tag="ps")
            nc.tensor.matmul(
                ps_g[:, 0, 0:1], lhsT=mconT, rhs=cum[:], start=True, stop=True
            )
            fail = scratch.tile([128, 1], f32, tag="fail")
            nc.vector.tensor_scalar(fail[:], ps_g[:, 0, 0:1], 0.0, None, ALU.is_gt)
            ps_nf = pp.tile([128, 2, 256], f32, tag="ps")
            nc.tensor.matmul(
                ps_nf[:, 0, 0:1], lhsT=onesJ, rhs=fail[:], start=True, stop=True
            )
            isok = scratch.tile([128, 1], f32, tag="isok")
            nc.vector.tensor_scalar(
                isok[:], ps_nf[:, 0, 0:1], 126.5, None, ALU.is_le
            )
            tm4 = scratch.tile([128, 1], f32, tag="tm4")
            nc.vector.tensor_scalar(
                tm4[:], ps_nf[:, 0, 0:1], 4.0, None, ALU.subtract
            )
            tsel = scratch.tile([128, 1], f32, tag="tsel")
            nc.vector.tensor_mul(tsel[:], tm4[:], isok[:])
            cutoffb = scratch.tile([128, 1], f32, tag="cutoffb")
            nc.vector.tensor_scalar(cutoffb[:], tsel[:], 5.0, None, ALU.add)
            inrowc = scratch.tile([128, 2], f32, tag="inrowc")
            nc.vector.tensor_scalar(
                inrowc[:], mrowc, cutoffb[:], None, ALU.is_le
            )

            # ====== build Csig = Er^T diag(w) Er + Ei^T diag(w) Ei ======
            # w[u] = inrow[(u+128)%256]: u-half 0 scales by inrow half 1.
            erw = scratch.tile([128, 2, N], f16, tag="erw")
            eiw = scratch.tile([128, 2, N], f16, tag="eiw")
            for hu in (0, 1):
                wsl = inrowc[:, 1 - hu : 2 - hu]
                nc.scalar.mul(erw[:, hu, :], er[:, hu, :], wsl)
                nc.vector.tensor_scalar(
                    eiw[:, hu, :], ei[:, hu, :], wsl, None, ALU.mult
                )
            csig = consts.tile([128, 2, N], f16, tag="csig")
            for hr in (0, 1):
                ps_c = pp.tile([128, 2, 256], f32, tag="ps")
                first = True
                for src in (erw, eiw):
                    base = er if src is erw else ei
                    for hu in (0, 1):
                        nc.tensor.matmul(
                            ps_c[:, 0, :],
                            lhsT=src[:, hu, ts(hr, 128)],
                            rhs=base[:, hu, :],
                            start=first,
                            stop=(src is eiw and hu == 1),
                        )
                        first = False
                nc.scalar.copy(csig[:, hr, :], ps_c[:, 0, :])

            # ============ main loop: out = |X~ - Csig X~ Csig| ============
            def stA(ch):
                """P = Csig @ X~ (both complex parts), psum [128, 2, 512]."""
                xw = x_tiles[ch // 2]
                c = 512 * (ch & 1)
                ps_p = pp.tile([128, 2, 512], f32, tag="ps")
                for m in (0, 1):
                    for part in (0, 1):
                        for hu in (0, 1):
                            nc.tensor.matmul(
                                ps_p[:, m, ds(256 * part, 256)],
                                lhsT=xw[:, hu, ds(c + 256 * part + m * 128, 128)],
                                rhs=csig[:, hu, :],
                                start=(hu == 0),
                                stop=(hu == 1),
                            )
                p16 = pp16.tile([128, 2, 512], f16, tag="p16")
                if ch % 4 == 3:
                    nc.vector.tensor_copy(p16[:], ps_p[:])
                else:
                    nc.scalar.copy(p16[:], ps_p[:])
                return p16

            o_tiles: dict[int, object] = {}

            def stB_abs(ch, p16):
                """Z = P @ Csig; out = sqrt((X~r-Zr)^2 + (X~i-Zi)^2)."""
                xw = x_tiles[ch // 2]
                c = 512 * (ch & 1)
                ps_q = pp.tile([128, 2, 512], f32, tag="ps")
                for my in (0, 1):
                    for part in (0, 1):
                        for mb in (0, 1):
                            nc.tensor.matmul(
                                ps_q[:, my, ds(256 * part, 256)],
                                lhsT=p16[:, mb, ds(256 * part + my * 128, 128)],
                                rhs=csig[:, mb, :],
                                start=(mb == 0),
                                stop=(mb == 1),
                            )
                a = sqp.tile([128, 2, N], f32, tag="a")
                nc.vector._custom_dve(
                    SQDIFF, out=a[:], in0=ps_q[:, :, 0:256],
                    in1=xw[:, :, ds(c, 256)], s0=1.0,
                )
                b = sqp.tile([128, 2, N], f32, tag="b")
                nc.vector._custom_dve(
                    SQDIFF, out=b[:], in0=ps_q[:, :, 256:512],
                    in1=xw[:, :, ds(c + 256, 256)], s0=1.0,
                )
                s = sqp.tile([128, 2, N], f32, tag="s")
                nc.gpsimd.tensor_add(s[:], a[:], b[:])
                if ch & 1 == 0:
                    ot = op.tile([128, 2, 2, N], f32, tag="o")
                    o_tiles[ch // 2] = ot
                o2 = o_tiles[ch // 2]
                nc.scalar.sqrt(o2[:, ch & 1, :, :], s[:])
                if ch & 1:
                    pr = ch // 2
                    x_tiles.pop(pr)
                    orows = out[2 * pr : 2 * pr + 2].rearrange(
                        "c (m p) x -> p c m x", p=128
                    )
                    nc.sync.dma_start(orows, o_tiles.pop(pr)[:])

            p16s: dict[int, object] = {}
            pfirst = stA(0)
            p16s[0] = pfirst
            for i in range(CH):
                if i % 2 == 0:
                    load_pair(i // 2 + 3)
                if i + 1 < CH:
                    pnext = stA(i + 1)
                    p16s[i + 1] = pnext
                stB_abs(i, p16s.pop(i))

    nc.compile()
    return nc


_CACHE: dict[str, object] = {}


def _get_nc():
    if "nc" not in _CACHE:
        _CACHE["nc"] = _build_nc()
    return _CACHE["nc"]


def _get_consts():
    if "consts" not in _CACHE:
        _CACHE["consts"] = _host_constants()
    return _CACHE["consts"]


def _run(x: np.ndarray, trace: bool = False):
    nc = _get_nc()
    consts = _get_consts()
    cph, sph = _host_phase_tables()
    in_maps = []
    for b in range(x.shape[0]):
        xb = np.asarray(x[b], dtype=np.float32)
        xtw = np.empty((CH, N, 512), dtype=np.float16)
        xtw[:, :, 0:256] = (xb * cph[None]).astype(np.float16)
        xtw[:, :, 256:512] = (xb * sph[None]).astype(np.float16)
        m = {
            "xtw": np.ascontiguousarray(
                xtw.reshape(CH // 2, 2, N, 512).transpose(0, 2, 1, 3).reshape(
                    CH // 2, N, 1024
                )
            ),
            "x0": xb[0].astype(np.float16),
        }
        m.update(consts)
        in_maps.append(m)
    res = run_bass_kernel_spmd(
        nc, in_maps, core_ids=list(range(len(in_maps))), trace=trace
    )
    out = np.stack([r["out"] for r in res.results]).astype(np.float32)
    return out, res


def kernel(x: np.ndarray) -> np.ndarray:
    x = np.asarray(x)
    out, _ = _run(x, trace=False)
    return out


# revision 12
# speedup vs baseline: 1.1148x; 1.0003x over previous
"""DHPF (dynamic high-pass filter) Trainium2 Bass kernel — Toeplitz v6.

Full inputs in, full outputs out. Sharding: pure data parallelism — sample b of
x[8, 64, 256, 256] goes to core b.

Algorithm (per core = 1 sample, 64 channels of 256x256):
  out = | X~ - Csig @ X~ @ Csig |,   X~ = X * e^{i pi (r+c)/256}
  with Csig[r,y] = sigma[y-r] real symmetric Toeplitz (the box-lowpass
  convolution operator with its rank-1 phase folded into the data; see v5).
  X~ is host-side input prep, shipped packed 2-channels-per-DMA. Csig is built
  on device once per sample from the channel-0 box-energy cutoff.
  abs() uses a custom DVE op SQDIFF_ANT: out = (in0-in1)^2.
  PE stream is software-pipelined (stA(i+1) before stB(i)); constants arrive
  in two packed DMAs; the cutoff scalar chain runs broadcast on [128,1] to
  avoid cross-engine round-trips.
"""

import sys
import types

import numpy as np

# The agent image's antenv is a stub without axon_hooks; rebuild the NTFF
# profile hook so trace=True (HW exec time) is available when requested.
try:
    if "antenv.axon_hooks" not in sys.modules:
        from trn_agent_boot.trn_boot import _ntff_profile_via_ctypes

        _hooks = types.ModuleType("antenv.axon_hooks")
        _h = _ntff_profile_via_ctypes("/opt/axon/libaxon_pjrt.so")
        _hooks.get_axon_ntff_profile_hook = lambda: _h
        _hooks.set_axon_ntff_profile_hook = lambda h: None
        sys.modules["antenv.axon_hooks"] = _hooks
except Exception:
    pass

import concourse.bass as bass
import concourse.tile as tile
from concourse import bacc, mybir
from concourse import bass_utils
from concourse.bass import ds, ts
from concourse.bass_utils import run_bass_kernel_spmd

try:
    bass_utils.upload_artifacts = lambda tmpdir: tmpdir
except Exception:
    pass

f32 = mybir.dt.float32
f16 = mybir.dt.float16
ALU = mybir.AluOpType

N = 256
CH = 64
ENERGY = 0.4


# ---------------- custom DVE op: out = (in0 - in1)^2 ----------------------
def _register_sqdiff():
    import concourse.dve_ops as dom
    from concourse.dve_spec import Spec, Src0, Src1, sq, lower, _has_src1
    from concourse.dve_uop import DveOpSpec

    name = "SQDIFF_ANT"
    for op in dom.OPS:
        if op.name == name:
            return op
    from concourse.dve_spec import C0
    spec = Spec(
        body=sq(Src0 * C0 - Src1),
        reference=lambda in0, in1, s0, s1, imm2: (
            (in0.astype(np.float32) * s0 - in1.astype(np.float32)) ** 2
        ).astype(np.float32),
    )
    opcode = dom._CUSTOM_DVE_ROW_BASE + len(dom.OPS)
    shas = {}
    for ver in ("v3", "v4"):
        try:
            d = DveOpSpec(
                name=name, opcode=opcode, uops=lower(spec, ver=ver),
                rd1_en=_has_src1(spec),
            )
            shas[ver] = d.sha(ver)
        except Exception:
            pass
    op = dom.DveOp(name, spec, subdim=False, uops_sha=shas)
    dom.OPS.append(op)
    dom.CUSTOM_DVE_SPECS[name] = spec
    dom._SUB_OPCODE_FOR_NAME[name] = opcode
    return op


SQDIFF = _register_sqdiff()


def _pack_rows(m):
    """[256, X] -> [128, 2X] in the _split layout (row r = i*128+p)."""
    return np.ascontiguousarray(
        np.stack([m[0:128], m[128:256]], axis=1).reshape(128, -1)
    )


def _host_constants() -> dict[str, np.ndarray]:
    u = np.arange(N)
    D = np.exp(-2j * np.pi * np.outer(u, u) / N)
    S = np.zeros((N, N))
    S[u, (u + N // 2) % N] = 1.0
    A = S @ D
    At = A.T  # [r, u]
    Atr, Ati = At.real, At.imag

    def pack(M1, M2, par):
        return np.concatenate(
            [M1[:128, par::2], M2[:128, par::2]], axis=1
        ).astype(np.float16)

    cabf = np.concatenate([Atr, Ati], axis=1)  # [256, 512]

    crow = N // 2
    dr = np.arange(N) - crow
    mr = np.maximum(-dr, dr + 1).astype(np.float64)
    cids = np.arange(128) + 1
    rmat = (mr[:, None] <= cids[None, :]).astype(np.float64)  # [256, 128]
    ctm = (mr[None, :] <= cids[:, None]).astype(np.float64)  # [128, 256]
    # scrambled-column version: col' = par*128 + j holds v = 2j + par
    ctmp = np.empty_like(ctm)
    jj = np.arange(128)
    for par in (0, 1):
        ctmp[:, par * 128 + jj] = ctm[:, 2 * jj + par]

    # g = mconT^T @ cum: g[p] = ENERGY*cum[127] - cum[p]
    mconT = -np.eye(128)
    mconT[127, :] += ENERGY
    onesJ = np.ones((128, 128))
    mrowc = np.stack([mr[0:128], mr[128:256]], axis=1)  # [128, 2]

    ph = np.pi * np.outer(2 * u + 1, np.arange(N)) / N
    er = np.cos(ph) / 16.0
    ei = -np.sin(ph) / 16.0

    cabfp = _pack_rows(cabf).astype(np.float16)  # [128, 1024]
    cf16 = np.concatenate(
        [
            pack(Atr, Ati, 0), pack(Atr, Ati, 1),
            pack(-Ati, Atr, 0), pack(-Ati, Atr, 1),  # 4 x [128, 256]
            _pack_rows(er),  # [128, 512]
            _pack_rows(ei),  # [128, 512]
            _pack_rows(rmat),  # [128, 256]
        ],
        axis=1,
    ).astype(np.float16)  # [128, 2304]
    cf32 = np.concatenate(
        [
            ctmp,  # [128, 256]
            mconT,  # [128, 128]
            onesJ,  # [128, 128]
            mrowc,  # [128, 2]
        ],
        axis=1,
    ).astype(np.float32)  # [128, 514]
    return {"cabfp": cabfp, "cf16": cf16, "cf32": cf32}


def _host_phase_tables():
    rc = np.pi * (np.arange(N)[:, None] + np.arange(N)[None, :]) / N
    return np.cos(rc).astype(np.float32), np.sin(rc).astype(np.float32)


def _split(t):
    """View a [256, X] dram AP as [128, 2, X] (partition, k-tile, free)."""
    return t.rearrange("(i p) j -> p i j", p=128)


def _build_nc():
    nc = bacc.Bacc("TRN2", target_bir_lowering=False, debug=False)

    xtw = nc.dram_tensor("xtw", [CH // 2, N, 1024], f16, kind="ExternalInput").ap()
    x0 = nc.dram_tensor("x0", [N, N], f16, kind="ExternalInput").ap()
    d_cabf = nc.dram_tensor("cabfp", [128, 1024], f16, kind="ExternalInput").ap()
    d_cf16 = nc.dram_tensor("cf16", [128, 2304], f16, kind="ExternalInput").ap()
    d_cf32 = nc.dram_tensor("cf32", [128, 514], f32, kind="ExternalInput").ap()
    out = nc.dram_tensor("out", [CH, N, N], f32, kind="ExternalOutput").ap()

    with tile.TileContext(nc) as tc:
        with (
            tc.tile_pool(name="consts", bufs=1) as consts,
            tc.tile_pool(name="xp_", bufs=6) as xpool,
            tc.tile_pool(name="pp16", bufs=6) as pp16,
            tc.tile_pool(name="sqp", bufs=6) as sqp,
            tc.tile_pool(name="op", bufs=4) as op,
            tc.tile_pool(name="scratch", bufs=1) as scratch,
            tc.tile_pool(name="pp", bufs=4, space="PSUM") as pp,
        ):
            # ---- gating DMAs first: x0, packed consts ----
            xz = scratch.tile([128, 2, N], f16, tag="xz")
            nc.sync.dma_start(xz[:], _split(x0))
            cabft = consts.tile([128, 1024], f16, tag="cabft")
            nc.sync.dma_start(cabft[:], d_cabf[:, :])
            cf16 = consts.tile([128, 2304], f16, tag="cf16")
            nc.sync.dma_start(cf16[:], d_cf16[:, :])
            cf32 = consts.tile([128, 514], f32, tag="cf32")
            nc.sync.dma_start(cf32[:], d_cf32[:, :])

            cabf = cabft[:].rearrange("p (i j) -> p i j", i=2)
            C16 = {
                "ab1e": cf16[:, ds(0, 256)],
                "ab1o": cf16[:, ds(256, 256)],
                "ab2e": cf16[:, ds(512, 256)],
                "ab2o": cf16[:, ds(768, 256)],
            }
            er = cf16[:, ds(1024, 512)].rearrange("p (i j) -> p i j", i=2)
            ei = cf16[:, ds(1536, 512)].rearrange("p (i j) -> p i j", i=2)
            rmat = cf16[:, ds(2048, 256)].rearrange("p (i j) -> p i j", i=2)
            ctmp = cf32[:, ds(0, 256)]
            mconT = cf32[:, ds(256, 128)]
            onesJ = cf32[:, ds(384, 128)]
            mrowc = cf32[:, ds(512, 2)]

            x_tiles: dict[int, object] = {}

            def load_pair(pr):
                if pr >= CH // 2:
                    return
                t = xpool.tile([128, 2, 1024], f16, tag="x")
                nc.sync.dma_start(t[:], _split(xtw[pr]))
                x_tiles[pr] = t

            for pr in range(3):
                load_pair(pr)

            # ---- PE warmup: ramp the clock while input DMAs are in flight
            warm = scratch.tile([128, 512], f16, tag="warm")
            nc.gpsimd.memset(warm[:], 0.0)
            zer4 = warm[:].rearrange("p (i j) -> p i j", i=4)
            ps_w = pp.tile([128, 2, 512], f32, tag="ps")
            for _ in range(30):
                nc.tensor.matmul(
                    ps_w[:, 0, 0:256], lhsT=warm[:, 0:128], rhs=warm[:, 0:256],
                    start=True, stop=True,
                )
            # ============ cutoff from channel 0 (parity forward DFT) ======
            ps1 = pp.tile([128, 2, 512], f32, tag="ps")
            for m in (0, 1):
                for k in (0, 1):
                    nc.tensor.matmul(
                        ps1[:, m, :],
                        lhsT=xz[:, k, ts(m, 128)],
                        rhs=cabf[:, k, :],
                        start=(k == 0),
                        stop=(k == 1),
                    )
            lo2 = scratch.tile([128, 512], f16, tag="utlo")
            nc.scalar.mul(lo2[:], ps1[:, 0, :], 2.0)
            utp = scratch.tile([128, 512], f16, tag="utp")
            nc.vector.scalar_tensor_tensor(
                out=utp[:], in0=lo2[:], scalar=0.5, in1=ps1[:, 1, :],
                op0=ALU.mult, op1=ALU.add,
            )
            utm = scratch.tile([128, 512], f16, tag="utm")
            nc.gpsimd.tensor_sub(utm[:], lo2[:], utp[:])

            ps0 = pp.tile([128, 4, 256], f32, tag="ps")
            for m in (0, 1):
                for par, src in ((0, utp), (1, utm)):
                    e = "e" if par == 0 else "o"
                    sl_re = src[:, ts(m, 128)]
                    sl_im = src[:, ds(256 + m * 128, 128)]
                    nc.tensor.matmul(
                        ps0[:, 2 * m + par, :], lhsT=sl_re, rhs=C16["ab1" + e],
                        start=True, stop=False,
                    )
                    nc.tensor.matmul(
                        ps0[:, 2 * m + par, :], lhsT=sl_im, rhs=C16["ab2" + e],
                        start=False, stop=True,
                    )

            # mag^2 directly on the (column-scrambled) spectrum psum
            mg1 = scratch.tile([128, 4, 128], f16, tag="mg1")
            nc.scalar.activation(
                mg1[:], ps0[:, :, 0:128],
                mybir.ActivationFunctionType.Square, 0.0, 1.0 / 128.0,
            )
            mg2 = scratch.tile([128, 4, 128], f16, tag="mg2")
            nc.vector._custom_dve(
                SQDIFF, out=mg2[:], in0=ps0[:, :, 128:256],
                in1=zer4, s0=1.0 / 128.0,
            )
            mag2 = scratch.tile([128, 4, 128], f16, tag="mag2")
            nc.gpsimd.tensor_add(mag2[:], mg1[:], mg2[:])

            ps_z = pp.tile([128, 2, 256], f32, tag="ps")
            for k in (0, 1):
                nc.tensor.matmul(
                    ps_z[:, 0, :], lhsT=rmat[:, k, :],
                    rhs=mag2[:, 2 * k : 2 * k + 2, :],
                    start=(k == 0), stop=(k == 1),
                )

            wsc = scratch.tile([128, N], f32, tag="wsc")
            cum = scratch.tile([128, 1], f32, tag="cum")
            nc.vector.scalar_tensor_tensor(
                out=wsc[:], in0=ps_z[:, 0, :], scalar=1.0, in1=ctmp,
                op0=ALU.mult, op1=ALU.mult, accum_out=cum[:],
            )
            # g[p] = ENERGY*cum[127] - cum[p]; fail = g > 0; nfb = sum(fail)
            ps_g = pp.tile([128, 2, 256], f32, tag="ps")
            nc.tensor.matmul(
                ps_g[:, 0, 0:1], lhsT=mconT, rhs=cum[:], start=True, stop=True
            )
            fail = scratch.tile([128, 1], f32, tag="fail")
            nc.vector.tensor_scalar(fail[:], ps_g[:, 0, 0:1], 0.0, None, ALU.is_gt)
            ps_nf = pp.tile([128, 2, 256], f32, tag="ps")
            nc.tensor.matmul(
                ps_nf[:, 0, 0:1], lhsT=onesJ, rhs=fail[:], start=True, stop=True
            )
            isok = scratch.tile([128, 1], f32, tag="isok")
            nc.vector.tensor_scalar(
                isok[:], ps_nf[:, 0, 0:1], 126.5, None, ALU.is_le
            )
            tm4 = scratch.tile([128, 1], f32, tag="tm4")
            nc.vector.tensor_scalar(
                tm4[:], ps_nf[:, 0, 0:1], 4.0, None, ALU.subtract
            )
            tsel = scratch.tile([128, 1], f32, tag="tsel")
            nc.vector.tensor_mul(tsel[:], tm4[:], isok[:])
            cutoffb = scratch.tile([128, 1], f32, tag="cutoffb")
            nc.vector.tensor_scalar(cutoffb[:], tsel[:], 5.0, None, ALU.add)
            inrowc = scratch.tile([128, 2], f32, tag="inrowc")
            nc.vector.tensor_scalar(
                inrowc[:], mrowc, cutoffb[:], None, ALU.is_le
            )

            # ====== build Csig = Er^T diag(w) Er + Ei^T diag(w) Ei ======
            # w[u] = inrow[(u+128)%256]: u-half 0 scales by inrow half 1.
            erw = scratch.tile([128, 2, N], f16, tag="erw")
            eiw = scratch.tile([128, 2, N], f16, tag="eiw")
            for hu in (0, 1):
                wsl = inrowc[:, 1 - hu : 2 - hu]
                nc.scalar.mul(erw[:, hu, :], er[:, hu, :], wsl)
                nc.vector.tensor_scalar(
                    eiw[:, hu, :], ei[:, hu, :], wsl, None, ALU.mult
                )
            csig = consts.tile([128, 2, N], f16, tag="csig")
            for hr in (0, 1):
                ps_c = pp.tile([128, 2, 256], f32, tag="ps")
                first = True
                for src in (erw, eiw):
                    base = er if src is erw else ei
                    for hu in (0, 1):
                        nc.tensor.matmul(
                            ps_c[:, 0, :],
                            lhsT=src[:, hu, ts(hr, 128)],
                            rhs=base[:, hu, :],
                            start=first,
                            stop=(src is eiw and hu == 1),
                        )
                        first = False
                nc.scalar.copy(csig[:, hr, :], ps_c[:, 0, :])

            # ============ main loop: out = |X~ - Csig X~ Csig| ============
            def stA(ch):
                """P = Csig @ X~ (both complex parts), psum [128, 2, 512]."""
                xw = x_tiles[ch // 2]
                c = 512 * (ch & 1)
                ps_p = pp.tile([128, 2, 512], f32, tag="ps")
                for m in (0, 1):
                    for part in (0, 1):
                        for hu in (0, 1):
                            nc.tensor.matmul(
                                ps_p[:, m, ds(256 * part, 256)],
                                lhsT=xw[:, hu, ds(c + 256 * part + m * 128, 128)],
                                rhs=csig[:, hu, :],
                                start=(hu == 0),
                                stop=(hu == 1),
                            )
                p16 = pp16.tile([128, 2, 512], f16, tag="p16")
                if ch % 4 == 3:
                    nc.vector.tensor_copy(p16[:], ps_p[:])
                else:
                    nc.scalar.copy(p16[:], ps_p[:])
                return p16

            o_tiles: dict[int, object] = {}

            def stB_abs(ch, p16):
                """Z = P @ Csig; out = sqrt((X~r-Zr)^2 + (X~i-Zi)^2)."""
                xw = x_tiles[ch // 2]
                c = 512 * (ch & 1)
                ps_q = pp.tile([128, 2, 512], f32, tag="ps")
                for my in (0, 1):
                    for part in (0, 1):
                        for mb in (0, 1):
                            nc.tensor.matmul(
                                ps_q[:, my, ds(256 * part, 256)],
                                lhsT=p16[:, mb, ds(256 * part + my * 128, 128)],
                                rhs=csig[:, mb, :],
                                start=(mb == 0),
                                stop=(mb == 1),
                            )
                a = sqp.tile([128, 2, N], f32, tag="a")
                nc.vector._custom_dve(
                    SQDIFF, out=a[:], in0=ps_q[:, :, 0:256],
                    in1=xw[:, :, ds(c, 256)], s0=1.0,
                )
                b = sqp.tile([128, 2, N], f32, tag="b")
                nc.vector._custom_dve(
                    SQDIFF, out=b[:], in0=ps_q[:, :, 256:512],
                    in1=xw[:, :, ds(c + 256, 256)], s0=1.0,
                )
                s = sqp.tile([128, 2, N], f32, tag="s")
                nc.gpsimd.tensor_add(s[:], a[:], b[:])
                if ch & 1 == 0:
                    ot = op.tile([128, 2, 2, N], f32, tag="o")
                    o_tiles[ch // 2] = ot
                o2 = o_tiles[ch // 2]
                nc.scalar.sqrt(o2[:, ch & 1, :, :], s[:])
                if ch & 1:
                    pr = ch // 2
                    x_tiles.pop(pr)
                    orows = out[2 * pr : 2 * pr + 2].rearrange(
                        "c (m p) x -> p c m x", p=128
                    )
                    nc.sync.dma_start(orows, o_tiles.pop(pr)[:])

            p16s: dict[int, object] = {}
            pfirst = stA(0)
            p16s[0] = pfirst
            for i in range(CH):
                if i % 2 == 0:
                    load_pair(i // 2 + 3)
                if i + 1 < CH:
                    pnext = stA(i + 1)
                    p16s[i + 1] = pnext
                stB_abs(i, p16s.pop(i))

    nc.compile()
    return nc


_CACHE: dict[str, object] = {}


def _get_nc():
    if "nc" not in _CACHE:
        _CACHE["nc"] = _build_nc()
    return _CACHE["nc"]


def _get_consts():
    if "consts" not in _CACHE:
        _CACHE["consts"] = _host_constants()
    return _CACHE["consts"]


def _run(x: np.ndarray, trace: bool = False):
    nc = _get_nc()
    consts = _get_consts()
    cph, sph = _host_phase_tables()
    in_maps = []
    for b in range(x.shape[0]):
        xb = np.asarray(x[b], dtype=np.float32)
        xtw = np.empty((CH, N, 512), dtype=np.float16)
        xtw[:, :, 0:256] = (xb * cph[None]).astype(np.float16)
        xtw[:, :, 256:512] = (xb * sph[None]).astype(np.float16)
        m = {
            "xtw": np.ascontiguousarray(
                xtw.reshape(CH // 2, 2, N, 512).transpose(0, 2, 1, 3).reshape(
                    CH // 2, N, 1024
                )
            ),
            "x0": xb[0].astype(np.float16),
        }
        m.update(consts)
        in_maps.append(m)
    res = run_bass_kernel_spmd(
        nc, in_maps, core_ids=list(range(len(in_maps))), trace=trace
    )
    out = np.stack([r["out"] for r in res.results]).astype(np.float32)
    return out, res


def kernel(x: np.ndarray) -> np.ndarray:
    x = np.asarray(x)
    out, _ = _run(x, trace=False)
    return out


# revision 13
# speedup vs baseline: 1.1187x; 1.0035x over previous
"""DHPF (dynamic high-pass filter) Trainium2 Bass kernel — Toeplitz v6.

Full inputs in, full outputs out. Sharding: pure data parallelism — sample b of
x[8, 64, 256, 256] goes to core b.

Algorithm (per core = 1 sample, 64 channels of 256x256):
  out = | X~ - Csig @ X~ @ Csig |,   X~ = X * e^{i pi (r+c)/256}
  with Csig[r,y] = sigma[y-r] real symmetric Toeplitz (the box-lowpass
  convolution operator with its rank-1 phase folded into the data; see v5).
  X~ is host-side input prep, shipped packed 2-channels-per-DMA. Csig is built
  on device once per sample from the channel-0 box-energy cutoff.
  abs() uses a custom DVE op SQDIFF_ANT: out = (in0-in1)^2.
  PE stream is software-pipelined (stA(i+1) before stB(i)); constants arrive
  in two packed DMAs; the cutoff scalar chain runs broadcast on [128,1] to
  avoid cross-engine round-trips.
"""

import sys
import types

import numpy as np

# The agent image's antenv is a stub without axon_hooks; rebuild the NTFF
# profile hook so trace=True (HW exec time) is available when requested.
try:
    if "antenv.axon_hooks" not in sys.modules:
        from trn_agent_boot.trn_boot import _ntff_profile_via_ctypes

        _hooks = types.ModuleType("antenv.axon_hooks")
        _h = _ntff_profile_via_ctypes("/opt/axon/libaxon_pjrt.so")
        _hooks.get_axon_ntff_profile_hook = lambda: _h
        _hooks.set_axon_ntff_profile_hook = lambda h: None
        sys.modules["antenv.axon_hooks"] = _hooks
except Exception:
    pass

import concourse.bass as bass
import concourse.tile as tile
from concourse import bacc, mybir
from concourse import bass_utils
from concourse.bass import ds, ts
from concourse.bass_utils import run_bass_kernel_spmd

try:
    bass_utils.upload_artifacts = lambda tmpdir: tmpdir
except Exception:
    pass

f32 = mybir.dt.float32
f16 = mybir.dt.float16
ALU = mybir.AluOpType

N = 256
CH = 64
ENERGY = 0.4


# ---------------- custom DVE op: out = (in0 - in1)^2 ----------------------
def _register_sqdiff():
    import concourse.dve_ops as dom
    from concourse.dve_spec import Spec, Src0, Src1, sq, lower, _has_src1
    from concourse.dve_uop import DveOpSpec

    name = "SQDIFF_ANT"
    for op in dom.OPS:
        if op.name == name:
            return op
    from concourse.dve_spec import C0
    spec = Spec(
        body=sq(Src0 * C0 - Src1),
        reference=lambda in0, in1, s0, s1, imm2: (
            (in0.astype(np.float32) * s0 - in1.astype(np.float32)) ** 2
        ).astype(np.float32),
    )
    opcode = dom._CUSTOM_DVE_ROW_BASE + len(dom.OPS)
    shas = {}
    for ver in ("v3", "v4"):
        try:
            d = DveOpSpec(
                name=name, opcode=opcode, uops=lower(spec, ver=ver),
                rd1_en=_has_src1(spec),
            )
            shas[ver] = d.sha(ver)
        except Exception:
            pass
    op = dom.DveOp(name, spec, subdim=False, uops_sha=shas)
    dom.OPS.append(op)
    dom.CUSTOM_DVE_SPECS[name] = spec
    dom._SUB_OPCODE_FOR_NAME[name] = opcode
    return op


SQDIFF = _register_sqdiff()


def _pack_rows(m):
    """[256, X] -> [128, 2X] in the _split layout (row r = i*128+p)."""
    return np.ascontiguousarray(
        np.stack([m[0:128], m[128:256]], axis=1).reshape(128, -1)
    )


def _host_constants() -> dict[str, np.ndarray]:
    u = np.arange(N)
    D = np.exp(-2j * np.pi * np.outer(u, u) / N)
    S = np.zeros((N, N))
    S[u, (u + N // 2) % N] = 1.0
    A = S @ D
    At = A.T  # [r, u]
    Atr, Ati = At.real, At.imag

    def pack(M1, M2, par):
        return np.concatenate(
            [M1[:128, par::2], M2[:128, par::2]], axis=1
        ).astype(np.float16)

    cabf = np.concatenate([Atr, Ati], axis=1)  # [256, 512]

    crow = N // 2
    dr = np.arange(N) - crow
    mr = np.maximum(-dr, dr + 1).astype(np.float64)
    cids = np.arange(128) + 1
    rmat = (mr[:, None] <= cids[None, :]).astype(np.float64)  # [256, 128]
    ctm = (mr[None, :] <= cids[:, None]).astype(np.float64)  # [128, 256]
    # scrambled-column version: col' = par*128 + j holds v = 2j + par
    ctmp = np.empty_like(ctm)
    jj = np.arange(128)
    for par in (0, 1):
        ctmp[:, par * 128 + jj] = ctm[:, 2 * jj + par]

    # g = mconT^T @ cum: g[p] = ENERGY*cum[127] - cum[p]
    mconT = -np.eye(128)
    mconT[127, :] += ENERGY
    onesJ = np.ones((128, 128))
    mrowc = np.stack([mr[0:128], mr[128:256]], axis=1)  # [128, 2]

    ph = np.pi * np.outer(2 * u + 1, np.arange(N)) / N
    er = np.cos(ph) / 16.0
    ei = -np.sin(ph) / 16.0

    cabfp = _pack_rows(cabf).astype(np.float16)  # [128, 1024]
    cf16 = np.concatenate(
        [
            pack(Atr, Ati, 0), pack(Atr, Ati, 1),
            pack(-Ati, Atr, 0), pack(-Ati, Atr, 1),  # 4 x [128, 256]
            _pack_rows(er),  # [128, 512]
            _pack_rows(ei),  # [128, 512]
            _pack_rows(rmat),  # [128, 256]
        ],
        axis=1,
    ).astype(np.float16)  # [128, 2304]
    cf32 = np.concatenate(
        [
            ctmp,  # [128, 256]
            mconT,  # [128, 128]
            onesJ,  # [128, 128]
            mrowc,  # [128, 2]
        ],
        axis=1,
    ).astype(np.float32)  # [128, 514]
    return {"cabfp": cabfp, "cf16": cf16, "cf32": cf32}


def _host_phase_tables():
    rc = np.pi * (np.arange(N)[:, None] + np.arange(N)[None, :]) / N
    return np.cos(rc).astype(np.float32), np.sin(rc).astype(np.float32)


def _split(t):
    """View a [256, X] dram AP as [128, 2, X] (partition, k-tile, free)."""
    return t.rearrange("(i p) j -> p i j", p=128)


def _build_nc():
    nc = bacc.Bacc("TRN2", target_bir_lowering=False, debug=False)

    xtw = nc.dram_tensor("xtw", [CH // 2, N, 1024], f16, kind="ExternalInput").ap()
    x0 = nc.dram_tensor("x0", [N, N], f16, kind="ExternalInput").ap()
    d_cabf = nc.dram_tensor("cabfp", [128, 1024], f16, kind="ExternalInput").ap()
    d_cf16 = nc.dram_tensor("cf16", [128, 2304], f16, kind="ExternalInput").ap()
    d_cf32 = nc.dram_tensor("cf32", [128, 514], f32, kind="ExternalInput").ap()
    out = nc.dram_tensor("out", [CH, N, N], f32, kind="ExternalOutput").ap()

    with tile.TileContext(nc) as tc:
        with (
            tc.tile_pool(name="consts", bufs=1) as consts,
            tc.tile_pool(name="xp_", bufs=6) as xpool,
            tc.tile_pool(name="pp16", bufs=6) as pp16,
            tc.tile_pool(name="sqp", bufs=6) as sqp,
            tc.tile_pool(name="op", bufs=4) as op,
            tc.tile_pool(name="scratch", bufs=1) as scratch,
            tc.tile_pool(name="pp", bufs=2, space="PSUM") as pp,
            tc.tile_pool(name="ppb", bufs=2, space="PSUM") as ppb,
        ):
            # ---- gating DMAs first: x0, packed consts ----
            xz = scratch.tile([128, 2, N], f16, tag="xz")
            nc.sync.dma_start(xz[:], _split(x0))
            cabft = consts.tile([128, 1024], f16, tag="cabft")
            nc.sync.dma_start(cabft[:], d_cabf[:, :])
            cf16 = consts.tile([128, 2304], f16, tag="cf16")
            nc.sync.dma_start(cf16[:], d_cf16[:, :])
            cf32 = consts.tile([128, 514], f32, tag="cf32")
            nc.sync.dma_start(cf32[:], d_cf32[:, :])

            cabf = cabft[:].rearrange("p (i j) -> p i j", i=2)
            C16 = {
                "ab1e": cf16[:, ds(0, 256)],
                "ab1o": cf16[:, ds(256, 256)],
                "ab2e": cf16[:, ds(512, 256)],
                "ab2o": cf16[:, ds(768, 256)],
            }
            er = cf16[:, ds(1024, 512)].rearrange("p (i j) -> p i j", i=2)
            ei = cf16[:, ds(1536, 512)].rearrange("p (i j) -> p i j", i=2)
            rmat = cf16[:, ds(2048, 256)].rearrange("p (i j) -> p i j", i=2)
            ctmp = cf32[:, ds(0, 256)]
            mconT = cf32[:, ds(256, 128)]
            onesJ = cf32[:, ds(384, 128)]
            mrowc = cf32[:, ds(512, 2)]

            x_tiles: dict[int, object] = {}

            def load_pair(pr):
                if pr >= CH // 2:
                    return
                t = xpool.tile([128, 2, 1024], f16, tag="x")
                nc.sync.dma_start(t[:], _split(xtw[pr]))
                x_tiles[pr] = t

            for pr in range(3):
                load_pair(pr)

            # ---- PE warmup: ramp the clock while input DMAs are in flight
            warm = scratch.tile([128, 512], f16, tag="warm")
            nc.gpsimd.memset(warm[:], 0.0)
            zer4 = warm[:].rearrange("p (i j) -> p i j", i=4)
            ps_w = pp.tile([128, 2, 512], f32, tag="ps")
            for _ in range(30):
                nc.tensor.matmul(
                    ps_w[:, 0, 0:256], lhsT=warm[:, 0:128], rhs=warm[:, 0:256],
                    start=True, stop=True,
                )
            # ============ cutoff from channel 0 (parity forward DFT) ======
            ps1 = pp.tile([128, 2, 512], f32, tag="ps")
            for m in (0, 1):
                for k in (0, 1):
                    nc.tensor.matmul(
                        ps1[:, m, :],
                        lhsT=xz[:, k, ts(m, 128)],
                        rhs=cabf[:, k, :],
                        start=(k == 0),
                        stop=(k == 1),
                    )
            lo2 = scratch.tile([128, 512], f16, tag="utlo")
            nc.scalar.mul(lo2[:], ps1[:, 0, :], 2.0)
            utp = scratch.tile([128, 512], f16, tag="utp")
            nc.vector.scalar_tensor_tensor(
                out=utp[:], in0=lo2[:], scalar=0.5, in1=ps1[:, 1, :],
                op0=ALU.mult, op1=ALU.add,
            )
            utm = scratch.tile([128, 512], f16, tag="utm")
            nc.gpsimd.tensor_sub(utm[:], lo2[:], utp[:])

            ps0 = pp.tile([128, 4, 256], f32, tag="ps")
            for m in (0, 1):
                for par, src in ((0, utp), (1, utm)):
                    e = "e" if par == 0 else "o"
                    sl_re = src[:, ts(m, 128)]
                    sl_im = src[:, ds(256 + m * 128, 128)]
                    nc.tensor.matmul(
                        ps0[:, 2 * m + par, :], lhsT=sl_re, rhs=C16["ab1" + e],
                        start=True, stop=False,
                    )
                    nc.tensor.matmul(
                        ps0[:, 2 * m + par, :], lhsT=sl_im, rhs=C16["ab2" + e],
                        start=False, stop=True,
                    )

            # mag^2 directly on the (column-scrambled) spectrum psum
            mg1 = scratch.tile([128, 4, 128], f16, tag="mg1")
            nc.scalar.activation(
                mg1[:], ps0[:, :, 0:128],
                mybir.ActivationFunctionType.Square, 0.0, 1.0 / 128.0,
            )
            mg2 = scratch.tile([128, 4, 128], f16, tag="mg2")
            nc.vector._custom_dve(
                SQDIFF, out=mg2[:], in0=ps0[:, :, 128:256],
                in1=zer4, s0=1.0 / 128.0,
            )
            mag2 = scratch.tile([128, 4, 128], f16, tag="mag2")
            nc.gpsimd.tensor_add(mag2[:], mg1[:], mg2[:])

            ps_z = pp.tile([128, 2, 256], f32, tag="ps")
            for k in (0, 1):
                nc.tensor.matmul(
                    ps_z[:, 0, :], lhsT=rmat[:, k, :],
                    rhs=mag2[:, 2 * k : 2 * k + 2, :],
                    start=(k == 0), stop=(k == 1),
                )

            wsc = scratch.tile([128, N], f32, tag="wsc")
            cum = scratch.tile([128, 1], f32, tag="cum")
            nc.vector.scalar_tensor_tensor(
                out=wsc[:], in0=ps_z[:, 0, :], scalar=1.0, in1=ctmp,
                op0=ALU.mult, op1=ALU.mult, accum_out=cum[:],
            )
            # g[p] = ENERGY*cum[127] - cum[p]; fail = g > 0; nfb = sum(fail)
            ps_g = pp.tile([128, 2, 256], f32, tag="ps")
            nc.tensor.matmul(
                ps_g[:, 0, 0:1], lhsT=mconT, rhs=cum[:], start=True, stop=True
            )
            fail = scratch.tile([128, 1], f32, tag="fail")
            nc.vector.tensor_scalar(fail[:], ps_g[:, 0, 0:1], 0.0, None, ALU.is_gt)
            ps_nf = pp.tile([128, 2, 256], f32, tag="ps")
            nc.tensor.matmul(
                ps_nf[:, 0, 0:1], lhsT=onesJ, rhs=fail[:], start=True, stop=True
            )
            isok = scratch.tile([128, 1], f32, tag="isok")
            nc.vector.tensor_scalar(
                isok[:], ps_nf[:, 0, 0:1], 126.5, None, ALU.is_le
            )
            tm4 = scratch.tile([128, 1], f32, tag="tm4")
            nc.vector.tensor_scalar(
                tm4[:], ps_nf[:, 0, 0:1], 4.0, None, ALU.subtract
            )
            tsel = scratch.tile([128, 1], f32, tag="tsel")
            nc.vector.tensor_mul(tsel[:], tm4[:], isok[:])
            cutoffb = scratch.tile([128, 1], f32, tag="cutoffb")
            nc.vector.tensor_scalar(cutoffb[:], tsel[:], 5.0, None, ALU.add)
            inrowc = scratch.tile([128, 2], f32, tag="inrowc")
            nc.vector.tensor_scalar(
                inrowc[:], mrowc, cutoffb[:], None, ALU.is_le
            )

            # ====== build Csig = Er^T diag(w) Er + Ei^T diag(w) Ei ======
            # w[u] = inrow[(u+128)%256]: u-half 0 scales by inrow half 1.
            erw = scratch.tile([128, 2, N], f16, tag="erw")
            eiw = scratch.tile([128, 2, N], f16, tag="eiw")
            for hu in (0, 1):
                wsl = inrowc[:, 1 - hu : 2 - hu]
                nc.scalar.mul(erw[:, hu, :], er[:, hu, :], wsl)
                nc.vector.tensor_scalar(
                    eiw[:, hu, :], ei[:, hu, :], wsl, None, ALU.mult
                )
            csig = consts.tile([128, 2, N], f16, tag="csig")
            for hr in (0, 1):
                ps_c = pp.tile([128, 2, 256], f32, tag="ps")
                first = True
                for src in (erw, eiw):
                    base = er if src is erw else ei
                    for hu in (0, 1):
                        nc.tensor.matmul(
                            ps_c[:, 0, :],
                            lhsT=src[:, hu, ts(hr, 128)],
                            rhs=base[:, hu, :],
                            start=first,
                            stop=(src is eiw and hu == 1),
                        )
                        first = False
                nc.scalar.copy(csig[:, hr, :], ps_c[:, 0, :])

            # ============ main loop: out = |X~ - Csig X~ Csig| ============
            def stA(ch):
                """P = Csig @ X~ (both complex parts), psum [128, 2, 512]."""
                xw = x_tiles[ch // 2]
                c = 512 * (ch & 1)
                ps_p = pp.tile([128, 2, 512], f32, tag="ps")
                for m in (0, 1):
                    for part in (0, 1):
                        for hu in (0, 1):
                            nc.tensor.matmul(
                                ps_p[:, m, ds(256 * part, 256)],
                                lhsT=xw[:, hu, ds(c + 256 * part + m * 128, 128)],
                                rhs=csig[:, hu, :],
                                start=(hu == 0),
                                stop=(hu == 1),
                            )
                p16 = pp16.tile([128, 2, 512], f16, tag="p16")
                if ch % 4 == 3:
                    nc.vector.tensor_copy(p16[:], ps_p[:])
                else:
                    nc.scalar.copy(p16[:], ps_p[:])
                return p16

            o_tiles: dict[int, object] = {}

            def stB_abs(ch, p16):
                """Z = P @ Csig; out = sqrt((X~r-Zr)^2 + (X~i-Zi)^2)."""
                xw = x_tiles[ch // 2]
                c = 512 * (ch & 1)
                ps_q = ppb.tile([128, 2, 512], f32, tag="psq")
                for my in (0, 1):
                    for part in (0, 1):
                        for mb in (0, 1):
                            nc.tensor.matmul(
                                ps_q[:, my, ds(256 * part, 256)],
                                lhsT=p16[:, mb, ds(256 * part + my * 128, 128)],
                                rhs=csig[:, mb, :],
                                start=(mb == 0),
                                stop=(mb == 1),
                            )
                a = sqp.tile([128, 2, N], f32, tag="a")
                nc.vector._custom_dve(
                    SQDIFF, out=a[:], in0=ps_q[:, :, 0:256],
                    in1=xw[:, :, ds(c, 256)], s0=1.0,
                )
                b = sqp.tile([128, 2, N], f32, tag="b")
                nc.vector._custom_dve(
                    SQDIFF, out=b[:], in0=ps_q[:, :, 256:512],
                    in1=xw[:, :, ds(c + 256, 256)], s0=1.0,
                )
                s = sqp.tile([128, 2, N], f32, tag="s")
                nc.gpsimd.tensor_add(s[:], a[:], b[:])
                if ch & 1 == 0:
                    ot = op.tile([128, 2, 2, N], f32, tag="o")
                    o_tiles[ch // 2] = ot
                o2 = o_tiles[ch // 2]
                nc.scalar.sqrt(o2[:, ch & 1, :, :], s[:])
                if ch & 1:
                    pr = ch // 2
                    x_tiles.pop(pr)
                    orows = out[2 * pr : 2 * pr + 2].rearrange(
                        "c (m p) x -> p c m x", p=128
                    )
                    nc.sync.dma_start(orows, o_tiles.pop(pr)[:])

            p16s: dict[int, object] = {}
            pfirst = stA(0)
            p16s[0] = pfirst
            for i in range(CH):
                if i % 2 == 0:
                    load_pair(i // 2 + 3)
                if i + 1 < CH:
                    pnext = stA(i + 1)
                    p16s[i + 1] = pnext
                stB_abs(i, p16s.pop(i))

    nc.compile()
    return nc


_CACHE: dict[str, object] = {}


def _get_nc():
    if "nc" not in _CACHE:
        _CACHE["nc"] = _build_nc()
    return _CACHE["nc"]


def _get_consts():
    if "consts" not in _CACHE:
        _CACHE["consts"] = _host_constants()
    return _CACHE["consts"]


def _run(x: np.ndarray, trace: bool = False):
    nc = _get_nc()
    consts = _get_consts()
    cph, sph = _host_phase_tables()
    in_maps = []
    for b in range(x.shape[0]):
        xb = np.asarray(x[b], dtype=np.float32)
        xtw = np.empty((CH, N, 512), dtype=np.float16)
        xtw[:, :, 0:256] = (xb * cph[None]).astype(np.float16)
        xtw[:, :, 256:512] = (xb * sph[None]).astype(np.float16)
        m = {
            "xtw": np.ascontiguousarray(
                xtw.reshape(CH // 2, 2, N, 512).transpose(0, 2, 1, 3).reshape(
                    CH // 2, N, 1024
                )
            ),
            "x0": xb[0].astype(np.float16),
        }
        m.update(consts)
        in_maps.append(m)
    res = run_bass_kernel_spmd(
        nc, in_maps, core_ids=list(range(len(in_maps))), trace=trace
    )
    out = np.stack([r["out"] for r in res.results]).astype(np.float32)
    return out, res


def kernel(x: np.ndarray) -> np.ndarray:
    x = np.asarray(x)
    out, _ = _run(x, trace=False)
    return out
